# revision 33
# baseline (speedup 1.0000x reference)
"""Trainium2 Bass kernel for nn_DeepseekOCRLayer (moe_routing).

Sharding (8 NeuronCores):
 - Attention: sequence-parallel over query blocks of 128 tokens; K/V computed
   replicated from full x (fp32 end-to-end: the router's top-6 selection is
   sensitive to ~1e-3 perturbations of h, so nothing upstream of the router
   may be quantized).  RMS scales are folded into rope tables / psum-copy
   scalars so the scaled activation tensor is never materialized; V is kept
   in natural layout so the per-head PV loop needs no V transposes; the Wo
   accumulation is folded into the per-head loop.
 - Router: computed per-core on own block in exact fp32 (softmax + top-6 +
   renorm) in natural expert order, then a tiny AllGather of the [T,64]
   routing-weight matrix; each core compacts its 8 expert columns via a
   per-core one-hot matmul.
 - h AllGathered once in bf16 (everything downstream of routing tolerates
   bf16).
 - MoE: 64 experts sharded 8/core; expert weights streamed in bf16 across
   three DMA queues (Act/SP/Pool), prefetch starting during attention /
   the collectives.  Token gather at capacity 128/expert via
   dma_gather(transpose=True) directly into the transposed layout the
   matmuls need; per-token routing weights gathered through a zero-padded
   table so unfilled slots self-mask; combine via dma_scatter_add + RS.
 - Shared experts: run as two full-width (896-wide) "pseudo-experts" over
   this core's own 128 tokens, through the same streamed-expert machinery;
   their output scatter-adds into this core's own block of the routed
   buffer, which the ReduceScatter then delivers (routed is zero-initialized
   on the idle Pool queue during attention).
"""

import numpy as np

H = 1280
T = 1024
NH = 10
HD = 128
EI = 896
NEXP = 64
TOPK = 6
SHF = 1792
NCORE = 8
P = 128
ELOC = NEXP // NCORE       # 8 experts per core
SHLOC = SHF // NCORE       # 224 shared-ffn cols per core
SHPAD = 256
CAP = 128
EPS = 1e-6
THETA = 10000.0
KC = H // P                # 10 contraction chunks
DK = EI // P               # 7 down-proj contraction chunks


def _build_nc():
    from contextlib import ExitStack
    import concourse.tile as tile
    from concourse import bacc, mybir

    f32 = mybir.dt.float32
    f32r = mybir.dt.float32r
    bf16 = mybir.dt.bfloat16
    i16 = mybir.dt.int16
    i32 = mybir.dt.int32
    AF = mybir.ActivationFunctionType
    OP = mybir.AluOpType
    AX = mybir.AxisListType

    nc = bacc.Bacc("TRN2", target_bir_lowering=False, debug=False,
                   num_devices=NCORE)

    def din(name, shape, dt=f32):
        return nc.dram_tensor(name, shape, dt, kind="ExternalInput").ap()

    x_my = din("x_my", [P, H])
    xT_my = din("xT_my", [H, P], f32r)
    xT_r = din("xT_r", [H, T], f32r)
    wq = din("wq", [H, H], f32r)
    wk = din("wk", [H, H], f32r)
    wv = din("wv", [H, H], f32r)
    wo = din("wo", [H, H], f32r)
    cosT = din("cosT", [HD, T])
    sinT = din("sinT", [HD, T])
    cosq = din("cosq", [P, HD])
    sinq = din("sinq", [P, HD])
    maskq = din("maskq", [P, T])
    gate_w = din("gate_w", [H, NEXP])
    eperm = din("eperm", [NEXP, ELOC])
    gpe = din("gpe", [ELOC, H, EI], bf16)
    upe = din("upe", [ELOC, H, EI], bf16)
    dpe = din("dpe", [ELOC, EI, H], bf16)
    shpe_g = din("shpe_g", [2, H, EI], bf16)
    shpe_u = din("shpe_u", [2, H, EI], bf16)
    shpe_d = din("shpe_d", [2, EI, H], bf16)
    myidx = din("myidx", [P, ELOC], mybir.dt.int16)
    ids_ones = din("ids_ones", [T, 2])
    ones1 = din("ones1", [1, P])
    onescol = din("onescol", [P, 1], f32r)
    zrowb = din("zrowb", [1, P], bf16)
    ones128 = din("ones128", [P, P])
    strictU = din("strictU", [P, P])
    iotaROW = din("iotaROW", [P, P])
    ident = din("ident", [P, P])
    identb = din("identb", [P, P], bf16)
    rotT = din("rotT", [P, P])

    out_my = nc.dram_tensor("out_my", [P, H], f32, kind="ExternalOutput").ap()

    with tile.TileContext(nc) as tc:
        with ExitStack() as main_ctx:
            const_pool = main_ctx.enter_context(
                tc.tile_pool(name="const", bufs=1))
            ident_sb = const_pool.tile([P, P], f32)
            nc.scalar.dma_start(ident_sb[:], ident[:])
            identb_sb = const_pool.tile([P, P], bf16)
            nc.scalar.dma_start(identb_sb[:], identb[:])
            ones1_sb = const_pool.tile([1, P], f32)
            nc.scalar.dma_start(ones1_sb[:], ones1[:])
            rotT_sb = const_pool.tile([P, P], f32)
            nc.scalar.dma_start(rotT_sb[:], rotT[:])
            ones_col = const_pool.tile([P, 1], f32r)
            nc.scalar.dma_start(ones_col[:], onescol[:])
            eps_tile = const_pool.tile([P, 1], f32)
            nc.vector.memset(eps_tile[:], EPS)
            eps1 = const_pool.tile([1, 1], f32)
            nc.vector.memset(eps1[:], EPS)
            zero_tile = const_pool.tile([P, 1], f32)
            nc.vector.memset(zero_tile[:], 0.0)
            zrow = const_pool.tile([1, P], bf16)
            nc.scalar.dma_start(zrow[:], zrowb[:])
            # routing consts - on Pool queue (idle early)
            ones_sb = const_pool.tile([P, P], f32)
            nc.gpsimd.dma_start(ones_sb[:], ones128[:])
            strU_sb = const_pool.tile([P, P], f32)
            nc.gpsimd.dma_start(strU_sb[:], strictU[:])
            iota_sb = const_pool.tile([P, P], f32)
            nc.gpsimd.dma_start(iota_sb[:], iotaROW[:])
            ion2 = const_pool.tile([P, NCORE, 2], f32)
            nc.gpsimd.dma_start(
                ion2[:], ids_ones[:].rearrange("(m p) c -> p m c", p=P))
            eperm_sb = const_pool.tile([NEXP, ELOC], f32)
            nc.gpsimd.dma_start(eperm_sb[:], eperm[:])
            onesf = const_pool.tile([P, 1], f32)
            nc.vector.memset(onesf[:], 1.0)
            myidx_sb = const_pool.tile([P, ELOC], i16)
            nc.gpsimd.dma_start(myidx_sb[:], myidx[:])

            keep_pool = main_ctx.enter_context(
                tc.tile_pool(name="keep", bufs=1))
            tmp_pool = main_ctx.enter_context(
                tc.tile_pool(name="tmp", bufs=2))
            dram = main_ctx.enter_context(
                tc.tile_pool(name="dram", bufs=1, space="DRAM"))

            h_my_sb = keep_pool.tile([P, H], f32)
            nc.vector.memset(h_my_sb[:], 0.0)
            s_cols = keep_pool.tile([P, NCORE], f32)
            Wfull_my = keep_pool.tile([P, NEXP], f32)
            Mall = keep_pool.tile([P, NCORE, ELOC], f32)
            Rp = keep_pool.tile([P, NCORE, ELOC], f32)
            idxs_all = keep_pool.tile([P, ELOC, ELOC], i16)
            sidxs_all = keep_pool.tile([P, ELOC, ELOC], i16)

            # expert weight stream ring on SP queue (top level: DMAs can be
            # issued during attention).  Half-matrix tiles to keep the SBUF
            # footprint low while attention is live.
            wpoolS = main_ctx.enter_context(
                tc.tile_pool(name="wpoolS", bufs=3))

            fx_dram = dram.tile([T, H], bf16)
            Wloc_dram = dram.tile([T + 1, P], bf16)
            routed = dram.tile([T + P, H], f32)
            idx_bounce = dram.tile([2 * ELOC, P], i16)
            for zb in range(9):
                nc.gpsimd.dma_start(routed[zb * P:(zb + 1) * P, :],
                                    h_my_sb[:])

            # ---------------- Phase A: attention (fp32) ----------------
            with ExitStack() as actx:
                apool = actx.enter_context(tc.tile_pool(name="apool", bufs=1))
                awork = actx.enter_context(tc.tile_pool(name="awork", bufs=2))
                kpool = actx.enter_context(tc.tile_pool(name="kpool", bufs=2))
                vpool = actx.enter_context(tc.tile_pool(name="vpool", bufs=2))
                wwork = actx.enter_context(tc.tile_pool(name="wwork", bufs=2))
                tmp1_pool = actx.enter_context(
                    tc.tile_pool(name="tmp1", bufs=1))

                ps_big = actx.enter_context(
                    tc.tile_pool(name="ps_big", bufs=1, space="PSUM"))
                ps_kt = actx.enter_context(
                    tc.tile_pool(name="ps_kt", bufs=1, space="PSUM"))
                ps_tr = actx.enter_context(
                    tc.tile_pool(name="ps_tr", bufs=2, space="PSUM"))
                ps_ot = actx.enter_context(
                    tc.tile_pool(name="ps_ot", bufs=1, space="PSUM"))

                xTr = apool.tile([P, KC, T], f32r)
                for c in range(KC):
                    nc.sync.dma_start(xTr[:, c, :],
                                      xT_r[c * P:(c + 1) * P, :])
                xTmy = apool.tile([P, KC, P], f32r)
                nc.sync.dma_start(
                    xTmy[:], xT_my[:].rearrange("(k p) j -> p k j", p=P))
                xmy_sb = apool.tile([P, H], f32)
                nc.gpsimd.dma_start(xmy_sb[:], x_my[:])

                cosT_sb = apool.tile([P, T], f32)
                nc.scalar.dma_start(cosT_sb[:], cosT[:])
                sinT_sb = apool.tile([P, T], f32)
                nc.scalar.dma_start(sinT_sb[:], sinT[:])
                cosq_sb = apool.tile([P, HD], f32)
                nc.scalar.dma_start(cosq_sb[:], cosq[:])
                sinq_sb = apool.tile([P, HD], f32)
                nc.scalar.dma_start(sinq_sb[:], sinq[:])
                maskq_sb = apool.tile([P, T], f32)
                nc.scalar.dma_start(maskq_sb[:], maskq[:])

                # rms scales for all tokens: ssq[t] = sum_h x[t,h]^2 via
                # ones-column matmul over xT chunks
                pss = ps_big.tile([1, T], f32, tag="bigp")
                for c in range(KC):
                    xsq = tmp1_pool.tile([P, T], f32r, tag="bigtmp")
                    nc.vector.tensor_tensor(out=xsq[:], in0=xTr[:, c, :],
                                            in1=xTr[:, c, :], op=OP.mult)
                    for n in range(2):
                        nc.tensor.matmul(pss[:, n * 512:(n + 1) * 512],
                                         ones_col[:],
                                         xsq[:, n * 512:(n + 1) * 512],
                                         start=(c == 0), stop=(c == KC - 1))
                srow_rt = tmp1_pool.tile([1, T], f32, tag="bigtmp")
                nc.scalar.activation(srow_rt[:], pss[:], AF.Sqrt,
                                     bias=eps1[:], scale=1.0 / H)
                S_rowf = apool.tile([1, T], f32)
                nc.vector.reciprocal(S_rowf[:], srow_rt[:])

                # per-block scale columns s_cols[:, m] = s[m*128 + p]
                for m in range(NCORE):
                    pt = ps_tr.tile([P, P], f32, tag="trp")
                    nc.tensor.transpose(pt[:, 0:1],
                                        S_rowf[:, m * P:(m + 1) * P],
                                        ident_sb[0:1, 0:1])
                    nc.vector.tensor_copy(s_cols[:, m:m + 1], pt[:, 0:1])

                # my-block rms scale (for the q side)
                s_my = tmp_pool.tile([P, 1], f32, tag="smy")
                xsq2 = tmp1_pool.tile([P, T], f32, tag="bigtmp")
                nc.vector.tensor_tensor(out=xsq2[:], in0=xmy_sb[:, 0:T],
                                        in1=xmy_sb[:, 0:T], op=OP.mult)
                xss = tmp_pool.tile([P, 1], f32, tag="xss")
                nc.vector.reduce_sum(out=xss[:], in_=xsq2[:], axis=AX.X)
                nc.vector.tensor_tensor(out=xsq2[:, 0:H - T],
                                        in0=xmy_sb[:, T:H],
                                        in1=xmy_sb[:, T:H], op=OP.mult)
                xs2 = tmp_pool.tile([P, 1], f32, tag="xs2")
                nc.vector.reduce_sum(out=xs2[:], in_=xsq2[:, 0:H - T],
                                     axis=AX.X)
                nc.vector.tensor_tensor(out=xss[:], in0=xss[:], in1=xs2[:],
                                        op=OP.add)
                xsr = tmp_pool.tile([P, 1], f32, tag="xsr")
                nc.scalar.activation(xsr[:], xss[:], AF.Sqrt,
                                     bias=eps_tile[:], scale=1.0 / H)
                nc.vector.reciprocal(s_my[:], xsr[:])

                # Q for my block (+rope; s_my and 1/sqrt(hd) folded into
                # the rope tables) -- transient psum scope
                pq = ps_big.tile([P, H], f32, tag="bigp")
                for k in range(KC):
                    wqk = wwork.tile([P, H], f32r, tag="wbuf")
                    nc.scalar.dma_start(wqk[:], wq[k * P:(k + 1) * P, :])
                    for n in range(3):
                        lo, hi = n * 512, min((n + 1) * 512, H)
                        nc.tensor.matmul(pq[:, lo:hi], xTmy[:, k, :],
                                         wqk[:, lo:hi],
                                         start=(k == 0), stop=(k == KC - 1))
                qrope = apool.tile([P, H], f32)
                cosq_s = tmp_pool.tile([P, HD], f32, tag="cqs")
                sinq_s = tmp_pool.tile([P, HD], f32, tag="sqs")
                nc.vector.tensor_scalar_mul(cosq_s[:], cosq_sb[:], s_my[:])
                nc.vector.tensor_scalar_mul(sinq_s[:], sinq_sb[:], s_my[:])
                for h in range(NH):
                    b = h * HD
                    t2 = tmp_pool.tile([P, 64], f32, tag="ropeq")
                    nc.vector.tensor_tensor(
                        out=qrope[:, b:b + 64], in0=pq[:, b:b + 64],
                        in1=cosq_s[:, :64], op=OP.mult)
                    nc.vector.tensor_tensor(
                        out=t2[:], in0=pq[:, b + 64:b + HD],
                        in1=sinq_s[:, :64], op=OP.mult)
                    nc.vector.tensor_tensor(
                        out=qrope[:, b:b + 64], in0=qrope[:, b:b + 64],
                        in1=t2[:], op=OP.subtract)
                    nc.vector.tensor_tensor(
                        out=qrope[:, b + 64:b + HD],
                        in0=pq[:, b + 64:b + HD],
                        in1=cosq_s[:, 64:], op=OP.mult)
                    nc.vector.tensor_tensor(
                        out=t2[:], in0=pq[:, b:b + 64],
                        in1=sinq_s[:, 64:], op=OP.mult)
                    nc.vector.tensor_tensor(
                        out=qrope[:, b + 64:b + HD],
                        in0=qrope[:, b + 64:b + HD], in1=t2[:], op=OP.add)

                # fold s into the k-side rope tables: cosT_s = cosT * s[t]
                B_s = tmp1_pool.tile([P, T], f32, tag="bigtmp")
                for n in range(2):
                    pb = ps_kt.tile([P, T], f32, tag="bps")
                    nc.tensor.matmul(pb[:, 0:512], ones1_sb[:],
                                     S_rowf[:, n * 512:(n + 1) * 512],
                                     start=True, stop=True)
                    nc.vector.tensor_copy(B_s[:, n * 512:(n + 1) * 512],
                                          pb[:, 0:512])
                nc.vector.tensor_tensor(out=cosT_sb[:], in0=cosT_sb[:],
                                        in1=B_s[:], op=OP.mult)
                nc.vector.tensor_tensor(out=sinT_sb[:], in0=sinT_sb[:],
                                        in1=B_s[:], op=OP.mult)

                # V in natural layout: build vT chunk, transpose into vnat
                vnat = apool.tile([P, NCORE, H], f32r)
                for c in range(KC):
                    wvc = wwork.tile([P, KC, P], f32r, tag="wbuf")
                    nc.sync.dma_start(
                        wvc[:], wv[:, c * P:(c + 1) * P].rearrange(
                            "(k p) j -> p k j", p=P))
                    vTc = vpool.tile([P, T], f32, tag="vtc")
                    pvt = ps_kt.tile([P, T], f32, tag="bps")
                    for n in range(2):
                        for k in range(KC):
                            nc.tensor.matmul(
                                pvt[:, n * 512:(n + 1) * 512], wvc[:, k, :],
                                xTr[:, k, n * 512:(n + 1) * 512],
                                start=(k == 0), stop=(k == KC - 1))
                    for n in range(2):
                        nc.vector.tensor_copy(vTc[:, n * 512:(n + 1) * 512],
                                              pvt[:, n * 512:(n + 1) * 512])
                    for m in range(NCORE):
                        pt = ps_tr.tile([P, P], f32, tag="trp")
                        nc.tensor.transpose(pt[:],
                                            vTc[:, m * P:(m + 1) * P],
                                            ident_sb[:])
                        nc.vector.tensor_copy(
                            vnat[:, m, c * P:(c + 1) * P], pt[:])

                # K^T chunk + head c, pipelined; Wo accumulated in-loop
                oT = apool.tile([P, NH, P], f32r)
                for c in range(KC):
                    wkc = wwork.tile([P, KC, P], f32r, tag="wbuf")
                    nc.sync.dma_start(
                        wkc[:], wk[:, c * P:(c + 1) * P].rearrange(
                            "(k p) j -> p k j", p=P))
                    kraw = awork.tile([P, T], f32, tag="kraw")
                    kTc = kpool.tile([P, T], f32r, tag="ktc")
                    t1 = tmp1_pool.tile([P, T], f32, tag="bigtmp")
                    pk = ps_kt.tile([P, T], f32, tag="bps")
                    for n in range(2):
                        for k in range(KC):
                            nc.tensor.matmul(
                                pk[:, n * 512:(n + 1) * 512], wkc[:, k, :],
                                xTr[:, k, n * 512:(n + 1) * 512],
                                start=(k == 0), stop=(k == KC - 1))
                    for n in range(2):
                        nc.vector.tensor_copy(kraw[:, n * 512:(n + 1) * 512],
                                              pk[:, n * 512:(n + 1) * 512])
                    prot = ps_kt.tile([P, T], f32, tag="bps")
                    for n in range(2):
                        nc.tensor.matmul(prot[:, n * 512:(n + 1) * 512],
                                         rotT_sb[:],
                                         kraw[:, n * 512:(n + 1) * 512],
                                         start=True, stop=True)
                    nc.vector.tensor_tensor(out=kTc[:], in0=kraw[:],
                                            in1=cosT_sb[:], op=OP.mult)
                    nc.vector.tensor_tensor(out=t1[:], in0=prot[:],
                                            in1=sinT_sb[:], op=OP.mult)
                    nc.vector.tensor_tensor(out=kTc[:], in0=kTc[:],
                                            in1=t1[:], op=OP.add)

                    pqt = ps_tr.tile([P, P], f32, tag="trp")
                    nc.tensor.transpose(pqt[:], qrope[:, c * HD:(c + 1) * HD],
                                        ident_sb[:])
                    qt_h = awork.tile([P, P], f32r, tag="qth")
                    nc.vector.tensor_copy(qt_h[:], pqt[:])
                    psc = ps_big.tile([P, T], f32, tag="bigp")
                    for n in range(2):
                        nc.tensor.matmul(psc[:, n * 512:(n + 1) * 512],
                                         qt_h[:],
                                         kTc[:, n * 512:(n + 1) * 512],
                                         start=True, stop=True)
                    nc.vector.tensor_tensor(out=psc[:], in0=psc[:],
                                            in1=maskq_sb[:], op=OP.add)
                    prob = awork.tile([P, T], f32, tag="kraw")
                    rsum = tmp_pool.tile([P, 1], f32, tag="rsum")
                    nc.scalar.activation(prob[:], psc[:], AF.Exp,
                                         bias=zero_tile[:], accum_out=rsum[:])
                    rinv = tmp_pool.tile([P, 1], f32, tag="rinv")
                    nc.vector.reciprocal(rinv[:], rsum[:])
                    pot = ps_ot.tile([P, P], f32, tag="otp")
                    for m in range(NCORE):
                        ppt = ps_tr.tile([P, P], f32, tag="trp")
                        nc.tensor.transpose(ppt[:],
                                            prob[:, m * P:(m + 1) * P],
                                            ident_sb[:])
                        ptk = awork.tile([P, P], f32r, tag="ptk")
                        # fold the v-side rms scale (per source token) into
                        # the psum->sbuf copy
                        nc.vector.tensor_scalar_mul(ptk[:], ppt[:],
                                                    s_cols[:, m:m + 1])
                        nc.tensor.matmul(pot[:],
                                         vnat[:, m, c * P:(c + 1) * P],
                                         ptk[:],
                                         start=(m == 0), stop=(m == NCORE - 1),
                                         skip_group_check=True)
                    nc.vector.tensor_copy(oT[:, c, :], pot[:])

                    woh = wwork.tile([P, H], f32r, tag="wbuf")
                    nc.scalar.dma_start(woh[:], wo[c * P:(c + 1) * P, :])
                    phc = ps_big.tile([P, H], f32, tag="bigp")
                    for n in range(3):
                        lo, hi = n * 512, min((n + 1) * 512, H)
                        nc.tensor.matmul(phc[:, lo:hi], oT[:, c, :],
                                         woh[:, lo:hi],
                                         start=True, stop=True)
                    nc.vector.scalar_tensor_tensor(
                        out=xmy_sb[:], in0=phc[:], scalar=rinv[:],
                        in1=xmy_sb[:], op0=OP.mult, op1=OP.add)
                nc.vector.tensor_copy(h_my_sb[:], xmy_sb[:])

            # ---------------- router (exact fp32, my block only) ----------
            hg_in = dram.tile([P, H + 80], bf16)
            with ExitStack() as rctx:
                rpool = rctx.enter_context(tc.tile_pool(name="rpool", bufs=1))
                rps = rctx.enter_context(
                    tc.tile_pool(name="rps", bufs=1, space="PSUM"))
                rpt = rctx.enter_context(
                    tc.tile_pool(name="rpt", bufs=2, space="PSUM"))

                gw = rpool.tile([P, KC, NEXP], f32)
                nc.scalar.dma_start(
                    gw[:], gate_w[:].rearrange("(k p) e -> p k e", p=P))

                hsq = rpool.tile([P, H], f32)
                nc.vector.tensor_tensor(out=hsq[:], in0=h_my_sb[:],
                                        in1=h_my_sb[:], op=OP.mult)
                hss = tmp_pool.tile([P, 1], f32, tag="hss")
                nc.vector.reduce_sum(out=hss[:], in_=hsq[:], axis=AX.X)
                hsr = tmp_pool.tile([P, 1], f32, tag="hsr")
                nc.scalar.activation(hsr[:], hss[:], AF.Sqrt,
                                     bias=eps_tile[:], scale=1.0 / H)
                sh_my = rpool.tile([P, 1], f32)
                nc.vector.reciprocal(sh_my[:], hsr[:])

                hT_my = rpool.tile([P, KC, P], f32)
                for k in range(KC):
                    pt = rpt.tile([P, P], f32, tag="rtr")
                    nc.tensor.transpose(pt[:], h_my_sb[:, k * P:(k + 1) * P],
                                        ident_sb[:])
                    nc.vector.tensor_copy(hT_my[:, k, :], pt[:])
                pr = rps.tile([P, NEXP], f32, tag="rps")
                for k in range(KC):
                    nc.tensor.matmul(pr[:], hT_my[:, k, :], gw[:, k, :],
                                     start=(k == 0), stop=(k == KC - 1))
                logit = rpool.tile([P, NEXP], f32)
                nc.vector.tensor_scalar_mul(logit[:], pr[:], sh_my[:])
                nmax = tmp_pool.tile([P, 1], f32, tag="rnmax")
                nc.vector.tensor_reduce(out=nmax[:], in_=logit[:],
                                        axis=AX.X, op=OP.max, negate=True)
                prob = rpool.tile([P, NEXP], f32)
                rsum = tmp_pool.tile([P, 1], f32, tag="rrsum")
                nc.scalar.activation(prob[:], logit[:], AF.Exp,
                                     bias=nmax[:], accum_out=rsum[:])
                rinv = tmp_pool.tile([P, 1], f32, tag="rrinv")
                nc.vector.reciprocal(rinv[:], rsum[:])
                nc.vector.tensor_scalar_mul(prob[:], prob[:], rinv[:])
                mx = tmp_pool.tile([P, 8], f32, tag="mx")
                nc.vector.max(mx[:], prob[:])
                nc.vector.memset(mx[:, TOPK:], -1.0)
                repl = rpool.tile([P, NEXP], f32)
                nc.vector.match_replace(repl[:], in_to_replace=mx[:],
                                        in_values=prob[:], imm_value=0.0)
                nc.vector.tensor_tensor(out=Wfull_my[:], in0=prob[:],
                                        in1=repl[:], op=OP.subtract)
                wsum = tmp_pool.tile([P, 1], f32, tag="wsum")
                nc.vector.reduce_sum(out=wsum[:], in_=Wfull_my[:], axis=AX.X)
                winv = tmp_pool.tile([P, 1], f32, tag="winv")
                nc.vector.reciprocal(winv[:], wsum[:])
                nc.vector.tensor_scalar_mul(Wfull_my[:], Wfull_my[:],
                                            winv[:])

                hgb = rpool.tile([P, H + 80], bf16)
                nc.vector.tensor_copy(hgb[:, 0:H], h_my_sb[:])
                nc.vector.tensor_copy(hgb[:, H:H + NEXP], Wfull_my[:])
                nc.vector.tensor_copy(hgb[:, H + NEXP:H + NEXP + 1],
                                      sh_my[:])
                nc.vector.tensor_copy(
                    hgb[:, H + NEXP + 1:],
                    zero_tile[:].to_broadcast([P, 80 - NEXP - 1]))
                nc.scalar.dma_start(hg_in[:], hgb[:])

            # ---------------- collective: [h | Wfull | sh] ----------------
            hg = dram.tile([T, H + 80], bf16, addr_space="Shared")
            nc.gpsimd.collective_compute(
                "AllGather", mybir.AluOpType.bypass,
                replica_groups=[list(range(NCORE))],
                ins=[hg_in[:]], outs=[hg[:]])

            # zero row so that unfilled slots (sidx=1024) gather w=0
            nc.scalar.dma_start(Wloc_dram[T:T + 1, :], zrow[:])

            # expert weight rings: Act queue prefetches expert 0 during the
            # collectives; Pool takes odd experts after the gathers.
            wpoolA = main_ctx.enter_context(
                tc.tile_pool(name="wpoolA", bufs=3))
            wpoolP = main_ctx.enter_context(
                tc.tile_pool(name="wpoolP", bufs=3))

            def unit_dram(u):
                kind, i = u
                if kind == "sh":
                    return ((shpe_g[i], EI, KC), (shpe_u[i], EI, KC),
                            (shpe_d[i], H, DK))
                return ((gpe[i], EI, KC), (upe[i], EI, KC), (dpe[i], H, DK))

            def load_full(u, pool, eng):
                mats = []
                for w_dram, csz, nch in unit_dram(u):
                    a = pool.tile([P, nch, csz], bf16, tag="wm")
                    eng.dma_start(a[:], w_dram[:].rearrange(
                        "(k p) j -> p k j", p=P))
                    mats.append((a, None, nch))
                return mats

            def s_half(u, mi, hi_half):
                w_dram, csz, nch = unit_dram(u)[mi]
                lo_n = nch // 2
                if not hi_half:
                    t = wpoolS.tile([P, lo_n, csz], bf16, tag="wm")
                    nc.sync.dma_start(
                        t[:], w_dram[0:lo_n * P, :].rearrange(
                            "(k p) j -> p k j", p=P))
                else:
                    t = wpoolS.tile([P, nch - lo_n, csz], bf16, tag="wm")
                    nc.sync.dma_start(
                        t[:], w_dram[lo_n * P:, :].rearrange(
                            "(k p) j -> p k j", p=P))
                return t

            def load_S(u):
                mats = []
                for mi, (_, _, nch) in enumerate(unit_dram(u)):
                    a = s_half(u, mi, False)
                    b = s_half(u, mi, True)
                    mats.append((a, b, nch // 2))
                return mats

            def mat_chunk(mat, k, lo, hi):
                a, b, lo_n = mat
                if b is None or k < lo_n:
                    return a[:, k, lo:hi]
                return b[:, k - lo_n, lo:hi]

            # consumption order: sh0, sh1, e0..e7
            # queues: A(Act): sh0, e0, e2   S(SP): sh1, e1, e4, e6
            #         P: e3 (via SP during the bubble), e5, e7 (via Pool)
            emats = {}
            emats[("sh", 0)] = load_full(("sh", 0), wpoolA, nc.scalar)
            # sh1 halves interleaved so the P-ring (e3) fills during the
            # collective bubble before SP hits a ring stall
            s1ga = s_half(("sh", 1), 0, False)
            s1gb = s_half(("sh", 1), 0, True)
            s1ua = s_half(("sh", 1), 1, False)
            emats[("ex", 3)] = load_full(("ex", 3), wpoolP, nc.sync)
            s1ub = s_half(("sh", 1), 1, True)
            s1da = s_half(("sh", 1), 2, False)
            s1db = s_half(("sh", 1), 2, True)
            emats[("sh", 1)] = [(s1ga, s1gb, KC // 2), (s1ua, s1ub, KC // 2),
                                (s1da, s1db, DK // 2)]
            emats[("ex", 1)] = load_S(("ex", 1))
            emats[("ex", 4)] = load_S(("ex", 4))
            emats[("ex", 6)] = load_S(("ex", 6))

            # ---------------- routing indices (during the h gather) -------
            with ExitStack() as ictx:
                ipool = ictx.enter_context(tc.tile_pool(name="ipool", bufs=1))
                iwork = ictx.enter_context(tc.tile_pool(name="iwork", bufs=2))
                ips = ictx.enter_context(
                    tc.tile_pool(name="ips", bufs=2, space="PSUM"))

                Wsbb = ipool.tile([P, NCORE, 80], bf16)
                nc.scalar.dma_start(
                    Wsbb[:], hg[:, H:].rearrange("(m p) e -> p m e", p=P))
                Wsb = keep_pool.tile([P, NCORE, 80], f32)
                nc.vector.tensor_copy(Wsb[:], Wsbb[:])

                # compact my 8 expert columns: Wmy[:, m, :] =
                #   (Wsb[:, m, :])^T^T @ eperm  via transpose + matmul
                Wmy = ipool.tile([P, NCORE, ELOC], f32)
                for m in range(NCORE):
                    ptw = ips.tile([P, P], f32, tag="ptw")
                    nc.tensor.transpose(ptw[0:NEXP, :], Wsb[:, m, 0:NEXP],
                                        ident_sb[:])
                    WsT = iwork.tile([NEXP, P], f32, tag="wst")
                    nc.vector.tensor_copy(WsT[:], ptw[0:NEXP, :])
                    pmy = ips.tile([P, ELOC], f32, tag="pmy")
                    nc.tensor.matmul(pmy[:], WsT[:], eperm_sb[:],
                                     start=True, stop=True)
                    nc.vector.tensor_copy(Wmy[:, m, :], pmy[:])

                nc.vector.tensor_scalar(out=Mall[:], in0=Wmy[:],
                                        scalar1=0.0, scalar2=None,
                                        op0=OP.is_gt)

                # prefix ranks r' (-1 for non-members)
                for i in range(NCORE):
                    prr = ips.tile([P, ELOC], f32, tag="prr")
                    for j in range(i + 1):
                        lhs = strU_sb if j == i else ones_sb
                        nc.tensor.matmul(prr[:], lhs[:], Mall[:, j, :],
                                         start=(j == 0), stop=(j == i))
                    rm = iwork.tile([P, ELOC], f32, tag="rm")
                    nc.vector.tensor_tensor(out=rm[:], in0=prr[:],
                                            in1=Mall[:, i, :], op=OP.mult)
                    nc.vector.tensor_tensor(out=rm[:], in0=rm[:],
                                            in1=Mall[:, i, :], op=OP.add)
                    nc.vector.tensor_scalar_add(Rp[:, i, :], rm[:], -1.0)

                # routed weights for my experts -> DRAM (for gathering)
                wlb = iwork.tile([P, NCORE, ELOC], bf16, tag="wlb")
                nc.vector.tensor_copy(wlb[:], Wmy[:])
                nc.scalar.dma_start(
                    Wloc_dram[0:T, 0:ELOC].rearrange("(m p) e -> p m e", p=P),
                    wlb[:])

                # slot -> token-id tables for all my experts
                for e in range(ELOC):
                    pid = ips.tile([P, 2], f32, tag="pid")
                    for i in range(NCORE):
                        se = iwork.tile([P, P], f32, tag="se")
                        nc.vector.tensor_tensor(
                            out=se[:],
                            in0=Rp[:, i, e:e + 1].to_broadcast([P, P]),
                            in1=iota_sb[:], op=OP.is_equal)
                        nc.tensor.matmul(pid[:], se[:], ion2[:, i, :],
                                         start=(i == 0), stop=(i == NCORE - 1))
                    idf = iwork.tile([P, 1], f32, tag="idf")
                    nc.vector.tensor_copy(idf[:], pid[:, 0:1])
                    idi = iwork.tile([P, 1], i32, tag="idi")
                    nc.vector.tensor_copy(idi[:], idf[:])
                    ids16 = iwork.tile([P, 1], i16, tag="ids16")
                    nc.vector.tensor_copy(ids16[:], idi[:])
                    sidf = iwork.tile([P, 1], f32, tag="sidf")
                    nc.vector.tensor_scalar_add(sidf[:], idf[:], -1024.0)
                    nc.vector.tensor_tensor(out=sidf[:], in0=sidf[:],
                                            in1=pid[:, 1:2], op=OP.mult)
                    nc.vector.tensor_scalar_add(sidf[:], sidf[:], 1024.0)
                    sidi = iwork.tile([P, 1], i32, tag="sidi")
                    nc.vector.tensor_copy(sidi[:], sidf[:])
                    sid16 = iwork.tile([P, 1], i16, tag="sid16")
                    nc.vector.tensor_copy(sid16[:], sidi[:])
                    nc.sync.dma_start(idx_bounce[e:e + 1, :], ids16[:, 0])
                    nc.sync.dma_start(idx_bounce[ELOC + e:ELOC + e + 1, :],
                                      sid16[:, 0])

                for rk in range(8):
                    nc.scalar.dma_start(
                        idxs_all[16 * rk:16 * (rk + 1), :, :],
                        idx_bounce[0:ELOC, :].rearrange(
                            "e (s p) -> p e s", p=16))
                    nc.scalar.dma_start(
                        sidxs_all[16 * rk:16 * (rk + 1), :, :],
                        idx_bounce[ELOC:2 * ELOC, :].rearrange(
                            "e (s p) -> p e s", p=16))

            # ---------------- fx (bf16) ------------------
            with ExitStack() as bctx:
                bwork = bctx.enter_context(tc.tile_pool(name="bwork", bufs=2))
                for m in range(NCORE):
                    hm = bwork.tile([P, H], bf16, tag="hm")
                    nc.sync.dma_start(hm[:], hg[m * P:(m + 1) * P, 0:H])
                    fxm = bwork.tile([P, H], bf16, tag="fxm")
                    nc.vector.tensor_scalar_mul(fxm[:], hm[:],
                                                Wsb[:, m, NEXP:NEXP + 1])

                    nc.sync.dma_start(fx_dram[m * P:(m + 1) * P, :], fxm[:])

            # Act-ring loads for e0/e2 (emitted after the fx writes so the
            # ring stall cannot head-block the routing/fx work on Act)
            emats[("ex", 0)] = load_full(("ex", 0), wpoolA, nc.scalar)
            emats[("ex", 2)] = load_full(("ex", 2), wpoolA, nc.scalar)

            # ------------- routed + shared experts (bf16) -------------
            with ExitStack() as ectx:
                ework = ectx.enter_context(tc.tile_pool(name="ework", bufs=2))
                egath = ectx.enter_context(tc.tile_pool(name="egath", bufs=2))
                epsg = ectx.enter_context(
                    tc.tile_pool(name="epsg", bufs=1, space="PSUM"))
                epsu = ectx.enter_context(
                    tc.tile_pool(name="epsu", bufs=1, space="PSUM"))
                epsy = ectx.enter_context(
                    tc.tile_pool(name="epsy", bufs=1, space="PSUM"))
                epst = ectx.enter_context(
                    tc.tile_pool(name="epst", bufs=1, space="PSUM"))

                UNITS = [("sh", 0), ("sh", 1)] + [("ex", e)
                                                  for e in range(ELOC)]

                def gather_unit(u):
                    kind, i = u
                    xeT = egath.tile([P, KC, P], bf16, tag="xeT")
                    idxap = myidx_sb[:] if kind == "sh" \
                        else idxs_all[:, i, :]
                    nc.gpsimd.dma_gather(
                        out_ap=xeT[:], in_ap=fx_dram[:], idxs_ap=idxap,
                        num_idxs=P, num_idxs_reg=P, elem_size=H,
                        transpose=True)
                    if kind == "sh":
                        return xeT, None
                    wsb = egath.tile([P, 1, P], bf16, tag="wsb")
                    nc.gpsimd.dma_gather(
                        out_ap=wsb[:], in_ap=Wloc_dram[:],
                        idxs_ap=sidxs_all[:, i, :],
                        num_idxs=P, num_idxs_reg=P, elem_size=P)
                    return xeT, wsb

                gathered = {UNITS[0]: gather_unit(UNITS[0])}

                for ui, u in enumerate(UNITS):
                    if u == ("ex", 3):
                        emats[("ex", 5)] = load_full(("ex", 5), wpoolP,
                                                     nc.gpsimd)
                    elif u == ("ex", 5):
                        emats[("ex", 7)] = load_full(("ex", 7), wpoolP,
                                                     nc.gpsimd)
                    if ui + 1 < len(UNITS):
                        gathered[UNITS[ui + 1]] = gather_unit(UNITS[ui + 1])
                    xeT, wsb = gathered.pop(u)
                    wg, wu, wd = emats.pop(u)
                    kind, i = u

                    if wsb is None:
                        wcol = onesf
                    else:
                        wcol = ework.tile([P, 1], f32, tag="wcol")
                        nc.vector.tensor_copy(wcol[:], wsb[:, 0, i:i + 1])

                    pg = epsg.tile([P, EI], f32, tag="epg")
                    for k in range(KC):
                        for n in range(2):
                            lo, hi = n * 512, min((n + 1) * 512, EI)
                            nc.tensor.matmul(
                                pg[:, lo:hi], xeT[:, k, :],
                                mat_chunk(wg, k, lo, hi),
                                start=(k == 0), stop=(k == KC - 1))
                    gsb = ework.tile([P, EI], f32, tag="gsb")
                    nc.scalar.activation(gsb[:], pg[:], AF.Sigmoid,
                                         bias=zero_tile[:])
                    nc.vector.tensor_tensor(out=gsb[:], in0=gsb[:],
                                            in1=pg[:], op=OP.mult)
                    pu = epsu.tile([P, EI], f32, tag="epu")
                    for k in range(KC):
                        for n in range(2):
                            lo, hi = n * 512, min((n + 1) * 512, EI)
                            nc.tensor.matmul(
                                pu[:, lo:hi], xeT[:, k, :],
                                mat_chunk(wu, k, lo, hi),
                                start=(k == 0), stop=(k == KC - 1))
                    zsb = ework.tile([P, EI], bf16, tag="zsb")
                    nc.vector.scalar_tensor_tensor(
                        out=zsb[:], in0=pu[:], scalar=wcol[:], in1=gsb[:],
                        op0=OP.mult, op1=OP.mult)
                    zT = ework.tile([P, DK, P], bf16, tag="zT")
                    for k in range(DK):
                        pt = epst.tile([P, P], bf16, tag="etr")
                        nc.tensor.transpose(pt[:], zsb[:, k * P:(k + 1) * P],
                                            identb_sb[:])
                        nc.vector.tensor_copy(zT[:, k, :], pt[:])
                    py = epsy.tile([P, H], f32, tag="epy")
                    for k in range(DK):
                        for n in range(3):
                            lo, hi = n * 512, min((n + 1) * 512, H)
                            nc.tensor.matmul(
                                py[:, lo:hi], zT[:, k, :],
                                mat_chunk(wd, k, lo, hi),
                                start=(k == 0), stop=(k == DK - 1))
                    ye = egath.tile([P, 1, H], f32, tag="ye")
                    nc.vector.tensor_copy(ye[:, 0, :], py[:])
                    nc.gpsimd.dma_scatter_add(
                        out_ap=routed[:], in_ap=ye[:],
                        idxs_ap=myidx_sb[:] if kind == "sh"
                        else sidxs_all[:, i, :],
                        num_idxs=P, num_idxs_reg=P, elem_size=H)

            # ---------------- combine ----------------
            rs_out = dram.tile([P, H], f32)
            nc.gpsimd.collective_compute(
                "ReduceScatter", mybir.AluOpType.add,
                replica_groups=[list(range(NCORE))],
                ins=[routed[0:T, :]], outs=[rs_out[:]])
            with ExitStack() as fctx:
                fpool = fctx.enter_context(tc.tile_pool(name="fpool", bufs=1))
                rsb = fpool.tile([P, H], f32)
                nc.scalar.dma_start(rsb[:], rs_out[:])
                osb = fpool.tile([P, H], f32)
                nc.vector.tensor_tensor(out=osb[:], in0=rsb[:],
                                        in1=h_my_sb[:], op=OP.add)
                nc.scalar.dma_start(out_my[:], osb[:])

    nc.compile()
    return nc


def host_inputs(inputs):
    """Prepare the 8 per-core input maps from the full problem inputs."""
    import ml_dtypes
    bf = ml_dtypes.bfloat16

    x = np.asarray(inputs["x"], np.float32).reshape(T, H)
    ln1 = np.asarray(inputs["ln1_w"], np.float32)
    ln2 = np.asarray(inputs["ln2_w"], np.float32)
    Wq = np.ascontiguousarray(np.asarray(inputs["Wq"], np.float32)
                              * ln1[:, None])
    Wk = np.ascontiguousarray(np.asarray(inputs["Wk"], np.float32)
                              * ln1[:, None])
    Wv = np.ascontiguousarray(np.asarray(inputs["Wv"], np.float32)
                              * ln1[:, None])
    Wo = np.asarray(inputs["Wo"], np.float32)
    gate_w = np.ascontiguousarray(
        np.asarray(inputs["gate_w"], np.float32) * ln2[:, None])
    gpe = np.asarray(inputs["gpe"], np.float32) * ln2[:, None, None]
    upe = np.asarray(inputs["upe"], np.float32) * ln2[:, None, None]
    dpe = np.asarray(inputs["dpe"], np.float32)
    shg = np.asarray(inputs["sh_gate"], np.float32) * ln2[:, None]
    shu = np.asarray(inputs["sh_up"], np.float32) * ln2[:, None]
    shd = np.asarray(inputs["sh_down"], np.float32)
    shpe_g = np.stack([shg[:, 0:EI], shg[:, EI:2 * EI]]).astype(bf)
    shpe_u = np.stack([shu[:, 0:EI], shu[:, EI:2 * EI]]).astype(bf)
    shpe_d = np.stack([shd[0:EI, :], shd[EI:2 * EI, :]]).astype(bf)

    xT = np.ascontiguousarray(x.T)
    inv = 1.0 / (THETA ** (np.arange(0, HD, 2, dtype=np.float32) / HD))
    f = inv[np.arange(HD) % 64].astype(np.float32)     # [128]
    tpos = np.arange(T, dtype=np.float32)
    ang = np.outer(f, tpos)                            # [128, 1024]
    cosT = np.cos(ang).astype(np.float32)
    sinT = np.sin(ang).astype(np.float32)
    sc = np.float32(1.0 / np.sqrt(HD))

    ids_ones = np.zeros((T, 2), np.float32)
    ids_ones[:, 0] = np.arange(T)
    ids_ones[:, 1] = 1.0
    ones1 = np.ones((1, P), np.float32)
    ones128 = np.ones((P, P), np.float32)
    strictU = np.triu(np.ones((P, P), np.float32), k=1)
    iotaROW = np.tile(np.arange(P, dtype=np.float32), (P, 1))
    ident = np.eye(P, dtype=np.float32)
    identb = np.eye(P, dtype=np.float32).astype(bf)
    rotT = np.zeros((P, P), np.float32)
    for j in range(64):
        rotT[j, j + 64] = 1.0
        rotT[j + 64, j] = -1.0

    maps = []
    for core in range(NCORE):
        tl = slice(core * P, (core + 1) * P)
        tg = np.arange(core * P, (core + 1) * P)
        angq = f[None, :] * tg[:, None].astype(np.float32)  # [128, 128]
        cosq = (np.cos(angq) * sc).astype(np.float32)
        sinq = (np.sin(angq) * sc).astype(np.float32)
        maskq = np.where(np.arange(T)[None, :] <= tg[:, None],
                         np.float32(0.0), np.float32(-1e30)).astype(np.float32)
        esl = slice(core * ELOC, (core + 1) * ELOC)
        epm = np.zeros((NEXP, ELOC), np.float32)
        for j in range(ELOC):
            epm[core * ELOC + j, j] = 1.0
        myidx = np.zeros((P, ELOC), np.int16)
        for p in range(P):
            for s in range(ELOC):
                myidx[p, s] = core * P + s * 16 + (p % 16)
        maps.append({
            "x_my": np.ascontiguousarray(x[tl]),
            "xT_my": np.ascontiguousarray(xT[:, tl]),
            "xT_r": xT,
            "wq": Wq, "wk": Wk, "wv": Wv, "wo": Wo,
            "cosT": cosT, "sinT": sinT,
            "cosq": cosq, "sinq": sinq,
            "maskq": np.ascontiguousarray(maskq),
            "gate_w": gate_w,
            "eperm": epm,
            "gpe": np.ascontiguousarray(
                gpe[:, :, esl].transpose(2, 0, 1)).astype(bf),
            "upe": np.ascontiguousarray(
                upe[:, :, esl].transpose(2, 0, 1)).astype(bf),
            "dpe": np.ascontiguousarray(
                dpe[:, :, esl].transpose(2, 0, 1)).astype(bf),
            "shpe_g": shpe_g, "shpe_u": shpe_u, "shpe_d": shpe_d,
            "myidx": myidx,
            "ids_ones": ids_ones,
            "ones1": ones1, "onescol": np.ones((P, 1), np.float32),
            "zrowb": np.zeros((1, P), bf),
            "ones128": ones128, "strictU": strictU,
            "iotaROW": iotaROW, "ident": ident, "identb": identb,
            "rotT": rotT,
        })
    return maps


_NC_CACHE = None
LAST_RESULT = None


def kernel(**inputs):
    global _NC_CACHE
    from concourse import bass_utils
    if _NC_CACHE is None:
        _NC_CACHE = _build_nc()
    maps = host_inputs(inputs)
    import os
    global LAST_RESULT
    try:
        res = bass_utils.run_bass_kernel_spmd(
            _NC_CACHE, maps, core_ids=list(range(NCORE)),
            trace=bool(os.environ.get("MOE_TRACE")))
    except ModuleNotFoundError:
        res = bass_utils.run_bass_kernel_spmd(
            _NC_CACHE, maps, core_ids=list(range(NCORE)))
    LAST_RESULT = res
    out = np.concatenate([res.results[i]["out_my"] for i in range(NCORE)],
                         axis=0)
    return out.reshape(1, T, H).astype(np.float32)
